# revision 4
# baseline (speedup 1.0000x reference)
"""ANI-2x NN potential (energy + forces), data-parallel across 8 NeuronCores.

Sharding: batch (molecule) axis B=64 -> 8 shards of 8 molecules, one per core.
AEV computation, species-routed MLPs, and the force autograd are independent
per molecule; MLP params are replicated to every core.

The graph is kept neuronx-cc-friendly: the only dot_generals are plain 2-D
MLP matmuls; every one-hot scatter/gather is expressed as masked reductions.
"""
import numpy as np
import jax
import jax.numpy as jnp

# ---- hardcoded model constants (match the ANI2xt reference) ----
RCR, RCA = 5.2, 3.5
ETAR, ZETA, ETAA = 16.0, 32.0, 8.0
SHFR = jnp.array([0.9, 1.16875, 1.4375, 1.70625, 1.975, 2.24375, 2.5125, 2.78125,
                  3.05, 3.31875, 3.5875, 3.85625, 4.125, 4.39375, 4.6625, 4.93125],
                 jnp.float32)
SHFZ = jnp.array([0.19634954, 0.58904862, 0.9817477, 1.3744468,
                  1.7671459, 2.1598449, 2.552544, 2.9452431], jnp.float32)
SHFA = jnp.array([0.9, 1.55, 2.2, 2.85], jnp.float32)
NSP = 7
NPAIR = NSP * (NSP + 1) // 2
SAE_NP = [-0.5984, -38.0826, -54.7031, -75.1901, -99.8006, -398.1224, -460.1387]
HARTREE2EV = 27.211386024367243
AEV_DIM = NSP * 16 + NPAIR * 32  # 1008

N_CORES = 8
B, A = 64, 32

# unordered pair index table (upper-triangle numbering, symmetric)
_PAIRS = []
_PT = np.zeros((NSP, NSP), np.int32)
_c = 0
for _s1 in range(NSP):
    for _s2 in range(_s1, NSP):
        _PT[_s1, _s2] = _PT[_s2, _s1] = _c
        _PAIRS.append((_s1, _s2))
        _c += 1


def fcut(r, rc):
    return jnp.where(r <= rc, 0.5 * jnp.cos(jnp.pi * r / rc) + 0.5, 0.0)


def compute_aev(sp_oh, coords):
    """sp_oh: [b,a,7] float one-hot of species; coords: [b,a,3]."""
    b, a, _ = coords.shape
    # G[b,i,j] = r_i . r_j via broadcasts (no batched dot_general)
    G = (coords[:, :, None, :] * coords[:, None, :, :]).sum(-1)   # [b,a,a]
    gd = (coords * coords).sum(-1)                                # [b,a]
    d2 = gd[:, :, None] + gd[:, None, :] - 2.0 * G + 1e-12
    dist = jnp.sqrt(d2)                                           # [b,a,a]
    not_self = 1.0 - jnp.eye(a, dtype=coords.dtype)
    # ---- radial ----
    rad = 0.25 * jnp.exp(-ETAR * (dist[..., None] - SHFR) ** 2) \
        * fcut(dist, RCR)[..., None] * not_self[None, :, :, None]  # [b,i,j,16]
    radial = []
    for s in range(NSP):
        m = sp_oh[:, None, :, s, None]                             # [b,1,j,1]
        radial.append((rad * m).sum(2))                            # [b,i,16]
    radial = jnp.stack(radial, 2).reshape(b, a, NSP * 16)          # [b,i,7*16]
    # ---- angular ----
    fc = fcut(dist, RCA) * not_self                                # [b,i,j]
    dot = (G[:, None, :, :] - G[:, :, :, None] - G[:, :, None, :]
           + gd[:, :, None, None])                                 # [b,i,j,k]
    cos_t = dot / (dist[:, :, :, None] * dist[:, :, None, :])
    ct = 0.95 * jnp.clip(cos_t, -1.0, 1.0)
    st = jnp.sqrt(1.0 - ct * ct)
    cos_tm = ct[..., None] * jnp.cos(SHFZ) + st[..., None] * jnp.sin(SHFZ)
    fz = ((1.0 + cos_tm) * 0.5) ** 32                              # [b,i,j,k,8]
    ravg = 0.5 * (dist[:, :, :, None] + dist[:, :, None, :])
    fe = jnp.exp(-ETAA * (ravg[..., None] - SHFA) ** 2)            # [b,i,j,k,4]
    tmask = fc[:, :, :, None] * fc[:, :, None, :] * not_self[None, None, :, :]
    term = (fz[..., :, None] * fe[..., None, :]).reshape(b, a, a, a, 32) \
        * tmask[..., None]                                         # [b,i,j,k,32]
    # stage 1: contract k with species masks -> U[sk][b,i,j,32]
    U = []
    for sk in range(NSP):
        mk = sp_oh[:, None, None, :, sk, None]                     # [b,1,1,k,1]
        U.append((term * mk).sum(3))                               # [b,i,j,32]
    # stage 2: contract j with species masks -> ang49[sj][sk] = [b,i,32]
    ang49 = [[None] * NSP for _ in range(NSP)]
    for sj in range(NSP):
        mj = sp_oh[:, None, :, sj, None]                           # [b,1,j,1]
        for sk in range(NSP):
            ang49[sj][sk] = (U[sk] * mj).sum(2)                    # [b,i,32]
    # fold ordered 49 -> unordered 28 pair channels
    ang = []
    for (s1, s2) in _PAIRS:
        if s1 == s2:
            ang.append(ang49[s1][s1])
        else:
            ang.append(ang49[s1][s2] + ang49[s2][s1])
    angular = jnp.stack(ang, 2).reshape(b, a, NPAIR * 32)
    return jnp.concatenate([radial, angular], -1)                  # [b,a,1008]


def celu(x):
    return jnp.maximum(x, 0.0) + jnp.minimum(0.0, 0.1 * (jnp.exp(x / 0.1) - 1.0))


def mlp2d(x, Ws, bs):
    n = len(Ws)
    for i in range(n):
        x = jnp.matmul(x, Ws[i]) + bs[i]
        if i < n - 1:
            x = celu(x)
    return x


def energy_fn(sp_oh, species_f, coords, params):
    b, a, _ = coords.shape
    aev = compute_aev(sp_oh, coords).reshape(b * a, AEV_DIM)
    atomic = jnp.zeros((b * a,), coords.dtype)
    sae = jnp.zeros((b, a), coords.dtype)
    for s in range(NSP):
        e_s = mlp2d(aev, params[s]['W'], params[s]['b'])[:, 0]     # [b*a]
        m = sp_oh[:, :, s].reshape(b * a)
        atomic = atomic + m * e_s
        sae = sae + sp_oh[:, :, s] * SAE_NP[s]
    energy = atomic.reshape(b, a).sum(-1) + sae.sum(-1)
    return energy * HARTREE2EV


def _shard_fn(sp_oh, species_f, coords, params):
    energy = energy_fn(sp_oh, species_f, coords, params)

    def esum(c):
        return energy_fn(sp_oh, species_f, c, params).sum()

    force = -jax.grad(esum)(coords)
    return energy, force


_PMAPPED = None


def _get_pmapped():
    global _PMAPPED
    if _PMAPPED is None:
        _PMAPPED = jax.pmap(_shard_fn, in_axes=(0, 0, 0, None),
                            devices=jax.devices()[:N_CORES])
    return _PMAPPED


def kernel(species, coords, params):
    species = np.asarray(species).astype(np.int32)
    coords = np.asarray(coords, dtype=np.float32)
    params = jax.tree_util.tree_map(lambda w: jnp.asarray(w, jnp.float32), params)

    # one-hot species on host (removes all integer ops/gathers from the graph)
    sp_oh = (species[..., None] == np.arange(NSP, dtype=np.int32)).astype(np.float32)

    sp_oh_sh = sp_oh.reshape(N_CORES, B // N_CORES, A, NSP)
    spf_sh = species.astype(np.float32).reshape(N_CORES, B // N_CORES, A)
    co_sh = coords.reshape(N_CORES, B // N_CORES, A, 3)

    energy, force = _get_pmapped()(sp_oh_sh, spf_sh, co_sh, params)
    energy = np.asarray(energy, dtype=np.float32).reshape(B)
    force = np.asarray(force, dtype=np.float32).reshape(B, A, 3)
    return energy, force
